# revision 1
# baseline (speedup 1.0000x reference)
"""Trainium2 Bass kernel for the exp-kernel multivariate Hawkes process
log-likelihood (B=8, N=2048, D=10).

Strategy
--------
Data-parallel over batch: core b computes batch row b fully on-chip and
returns one scalar; the host gathers the 8 scalars.

Per core the O(N^2) pairwise interaction is restructured into a chunked
O(N*D^2) algorithm (chunk size C=128 = partition count). Over (r,m) =
(receiver, trigger) type pairs (RM=100), with per-chunk reference times
ts_k:

  W[j,(r,m)]  = [e_j==m] * exp( b[r,m] (t_j - ts_k))
  P           = inclusive prefix of W over j within the chunk
                (PE matmul with upper-triangular ones)
  S_k[(r,m)]  = sum_{j < chunk k} exp(-b[r,m](ts_k - t_j))
                (inter-chunk state; affine scan over chunks)
  lam_i       = musub[e_i] + sum_{r,m} [e_i==r] exp(-b(t_i-ts)) ab[r,m] (P+S)[i,(r,m)]

The inclusive prefix counts the self pair j==i contributing exactly
ab[e_i,e_i]; host-precomputed musub = mu - diag(ab) cancels it.

The inter-chunk recurrence S_{k+1} = d_k*(S_k + Wsum_k) is ONE
`tensor_tensor_scan` in transposed layout [100,16]; per-chunk column
sums come from N=1 matmuls, and S is broadcast into PSUM with K=1
matmuls against a flattened S row.

The integral term uses the same masking trick with transposed tables:
  neg_ev_j = sum_m onehot[j,m] (asumT[m] - sum_d aT[m,d] exp(bT[m,d](t_j-T)))

Precision: exp arguments and all accumulations are fp32; post-exp
values, 0/1 masks, and matmul operands are bf16 (DVE 2x mode + PE
single-pass). Host-side work is limited to O(D^2) parameter softplus,
O(N) reshapes and the 16 chunk reference times.
"""
import numpy as np
from contextlib import ExitStack

import ml_dtypes
import concourse.bass as bass
import concourse.mybir as mybir
import concourse.tile as tile
from concourse import bacc
from concourse.bass_utils import run_bass_kernel_spmd

f32 = mybir.dt.float32
bf16 = mybir.dt.bfloat16
AL = mybir.AluOpType
AF = mybir.ActivationFunctionType
AX = mybir.AxisListType

P = 128          # partitions == chunk size
KC = 16          # number of chunks
D = 10           # event types
RM = D * D       # (receiver, trigger) pairs
N = P * KC       # 2048 events per batch row
B = 8            # batch == cores
NG = 4           # chunk groups (4 chunks per PSUM bank)

# packed DRAM inputs: name -> (shape, dtype)
INPUTS = {
    "pack_f32": ((P, 285), f32),    # t(16) e(16) tstart(16) bflat(100)
                                    # musub(10) asumT(10) ident(100)
                                    # bdtb(16) negconst(1)
    "pack_bf": ((P, 748), bf16),    # triu(128) abflat(100) aTflat(100)
                                    # bTflat(100) iota(160) onehot(160)
    "oht": ((D, N + 23), bf16),     # onehotT | [bT aT musub asum_hi asum_lo]
    "selmask": ((KC, KC * RM), bf16),  # selmask[k, kk*100+rm] = [k == kk-1]
}


def _body(ctx: ExitStack, tc, ins, out_ap, Tval: float):
    nc = tc.nc
    cpool = ctx.enter_context(tc.tile_pool(name="cpool", bufs=1))
    wpool = ctx.enter_context(tc.tile_pool(name="wpool", bufs=1))
    spool = ctx.enter_context(tc.tile_pool(name="spool", bufs=1))
    pp = ctx.enter_context(tc.tile_pool(name="pp", bufs=1, space="PSUM"))
    ps = ctx.enter_context(tc.tile_pool(name="ps", bufs=1, space="PSUM"))

    # ---- load packed inputs on two parallel DMA queues ----
    pf = cpool.tile([P, 285], f32, tag="pf")
    nc.sync.dma_start(out=pf[:, 0:148], in_=ins["pack_f32"][:, 0:148])
    pb = cpool.tile([P, 748], bf16, tag="pb")
    nc.sync.dma_start(out=pb[:], in_=ins["pack_bf"])
    nc.sync.dma_start(out=pf[:, 148:], in_=ins["pack_f32"][:, 148:])
    oht = cpool.tile([D, N + 23], bf16, tag="oht")
    nc.scalar.dma_start(out=oht[:], in_=ins["oht"])
    selmask = cpool.tile([KC, KC * RM], bf16, tag="selmask")
    nc.gpsimd.dma_start(out=selmask[:], in_=ins["selmask"])

    trel_in = pf[:, 0:16]
    e128 = pf[:, 16:32]
    tau2_in = pf[:, 32:48]
    bflat = pf[:, 48:148].rearrange("p (r m) -> p r m", r=D)
    musub = pf[:, 148:158]
    asumT = pf[:, 158:168]
    ident = pf[0:RM, 168:268]
    bdtb = pf[0:RM, 268:284]
    negconst = pf[0:1, 284:285]
    triu = pb[:, 0:128]
    abflat = pb[:, 128:228].rearrange("p (r m) -> p r m", r=D)
    aTflat = pb[:, 228:328].rearrange("p (m d) -> p m d", m=D)
    bTflat = pb[:, 328:428].rearrange("p (m d) -> p m d", m=D)
    iota10 = pb[:, 428:588]

    # ---- constants ----
    ones16 = cpool.tile([KC, P], bf16, tag="ones16")
    nc.vector.memset(ones16[:], 1.0)
    ones_col = cpool.tile([P, 1], f32, tag="ones_col")
    nc.vector.memset(ones_col[:], 1.0)
    ones_col_bf = cpool.tile([P, 1], bf16, tag="ones_col_bf")
    nc.vector.memset(ones_col_bf[:], 1.0)

    # ---- per-event scalars ----
    onehot = pb[:, 588:748].rearrange("p (k d) -> p k d", k=KC)
    trel = trel_in
    tau2 = tau2_in

    # ---- positive-part exp pipeline (per group, so the PE starts early) ----
    argW = wpool.tile([P, KC, D, D], f32, tag="argW")
    expW = wpool.tile([P, KC, D, D], bf16, tag="expW")
    expU = wpool.tile([P, KC, D, D], bf16, tag="expU")
    for g in range(NG):
        gs = slice(4 * g, 4 * (g + 1))
        nc.vector.tensor_tensor(
            out=argW[:, gs],
            in0=trel[:, gs].unsqueeze(2).unsqueeze(3)
                .broadcast_to([P, 4, D, D]),
            in1=bflat.unsqueeze(1).broadcast_to([P, 4, D, D]),
            op=AL.mult)
        nc.scalar.activation(expW[:, gs], argW[:, gs], AF.Exp)

    nc.scalar.activation(expU[:], argW[:], AF.Exp, scale=-1.0)

    # W = expW * onehot[m]; all-bf16 SBUF => DVE 2x mode, per group for
    # PE overlap
    W = wpool.tile([P, KC, D, D], bf16, tag="W")
    for g in range(NG):
        gs = slice(4 * g, 4 * (g + 1))
        nc.vector.tensor_tensor(
            out=W[:, gs], in0=expW[:, gs],
            in1=onehot[:, gs].unsqueeze(2).broadcast_to([P, 4, D, D]),
            op=AL.mult)
    # expUab = exp(-argW) * ab  (all-bf16, 2x; per group to fill DVE gaps)
    expUab = wpool.tile([P, KC, D, D], bf16, tag="expUab")
    for g in range(NG):
        gs = slice(4 * g, 4 * (g + 1))
        nc.vector.tensor_tensor(
            out=expUab[:, gs], in0=expU[:, gs],
            in1=abflat.unsqueeze(1).broadcast_to([P, 4, D, D]), op=AL.mult)


    # ---- PE phase ----
    Pg = [pp.tile([P, 4, D, D], f32, tag=f"Pg{g}", name=f"Pg{g}")
          for g in range(NG)]
    wsumc = ps.tile([RM, KC], f32, tag="wsumc")
    for k in range(KC):
        nc.tensor.matmul(wsumc[:, k:k + 1],
                         W[:, k].rearrange("p r m -> p (r m)"),
                         ones_col_bf[:], start=True, stop=True)

    # ---- per-event gathered tables: grows[:,k,:] = onehotT_k^T @ tabs ----
    # cols: bT-row(10) | aT-row(10) | musub[e](1) | asumT[e](1)
    grows = ps.tile([P, KC, 23], f32, tag="grows")
    for k in range(KC):
        nc.tensor.matmul(grows[:, k], oht[:, k * P:(k + 1) * P],
                         oht[:, N:N + 23], start=True, stop=True)

    # ---- negative (integral) part via gathered per-event rows ----
    argN2 = wpool.tile([P, KC, D], f32, tag="argN2")
    nc.vector.tensor_tensor(
        out=argN2[:], in0=grows[:, :, 0:10],
        in1=tau2[:].unsqueeze(2).broadcast_to([P, KC, D]), op=AL.mult)
    expN2 = wpool.tile([P, KC, D], bf16, tag="expN2")
    nc.scalar.activation(expN2[:], argN2[:], AF.Exp)
    nmul = wpool.tile([P, KC, D], f32, tag="nmul")
    nc.vector.tensor_tensor(out=nmul[:], in0=expN2[:], in1=grows[:, :, 10:20],
                            op=AL.mult)
    negsub = wpool.tile([P, KC], f32, tag="negsub")
    nc.vector.tensor_reduce(out=negsub[:], in_=nmul[:], axis=AX.X, op=AL.add)
    ngt1 = wpool.tile([P, KC], f32, tag="ngt1")
    nc.vector.tensor_tensor(out=ngt1[:], in0=negsub[:], in1=grows[:, :, 21],
                            op=AL.subtract)
    negtot = wpool.tile([P, KC], f32, tag="negtot")
    nc.vector.tensor_tensor(out=negtot[:], in0=ngt1[:], in1=grows[:, :, 22],
                            op=AL.subtract)





    # ---- inter-chunk affine scan (transposed layout [100,16]) ----
    decayT = spool.tile([RM, KC], f32, tag="decayT")
    nc.scalar.activation(decayT[:], bdtb, AF.Exp, scale=-1.0)
    SCOL = spool.tile([RM, KC], f32, tag="SCOL")
    nc.vector.tensor_tensor_scan(SCOL[:], wsumc[:], decayT[:], initial=0.0,
                                 op0=AL.add, op1=AL.mult)
    # SCOL[:, t] = S_{t+1}; transpose and flatten to a partition-0 row
    # (matmul operands must be quadrant-aligned), block 0 = S_0 = 0
    stp = ps.tile([KC, RM], f32, tag="stp")
    nc.tensor.transpose(stp[:], SCOL[:], ident)
    srows = spool.tile([KC, RM], bf16, tag="srows")
    nc.vector.tensor_copy(out=srows[:], in_=stp[:])
    # rhs_all[k, (kk,rm)] = S_{k+1}[rm] * [k == kk-1]; summing over k in the
    # K=16 inject matmul yields S_kk per chunk block (zero for kk=0)
    rhs_all = spool.tile([KC, KC, RM], bf16, tag="rhs_all")
    for g in range(NG):
        gs = slice(4 * g, 4 * (g + 1))
        nc.vector.tensor_tensor(
            out=rhs_all[:, gs],
            in0=srows[:].unsqueeze(1).broadcast_to([KC, 4, RM]),
            in1=selmask[:].rearrange("k (c rm) -> k c rm", rm=RM)[:, gs],
            op=AL.mult)

    # batched inclusive prefix (fills the PE while the S row is being built)
    for g in range(NG):
        nc.tensor.matmul(Pg[g][:],
                         triu,
                         W[:, 4 * g:4 * (g + 1)].rearrange(
                             "p c r m -> p (c r m)"),
                         start=True, stop=False)

    # batched S inject: one K=16 matmul per group broadcasts S_k to all rows
    for g in range(NG):
        nc.tensor.matmul(Pg[g][:], ones16[:],
                         rhs_all[:, 4 * g:4 * (g + 1)].rearrange(
                             "k c rm -> k (c rm)"),
                         start=False, stop=True)

    # ---- positive part: lam via fused multiply-reduce per chunk ----
    # lam[:,k] = musub_ev[:,k] + sum_rm U2ab[:,k,rm] * (P+S)[:,k,rm]
    lamr = wpool.tile([P, KC], f32, tag="lamr")
    PM = wpool.tile([P, KC, D, D], bf16, tag="PM")
    G2 = wpool.tile([P, KC, D, D], bf16, tag="G2")
    for g in range(NG):
        gs = slice(4 * g, 4 * (g + 1))
        nc.vector.tensor_tensor(
            out=PM[:, gs], in0=Pg[g][:],
            in1=onehot[:, gs].unsqueeze(3).broadcast_to([P, 4, D, D]),
            op=AL.mult)
        nc.vector.tensor_tensor(out=G2[:, gs], in0=PM[:, gs],
                                in1=expUab[:, gs], op=AL.mult)
        nc.vector.tensor_reduce(
            out=lamr[:, gs],
            in_=G2[:, gs].rearrange("p c r m -> p c (r m)"),
            axis=AX.X, op=AL.add)
    lam = wpool.tile([P, KC], f32, tag="lam")
    nc.vector.tensor_tensor(out=lam[:], in0=lamr[:], in1=grows[:, :, 20],
                            op=AL.add)
    loglam = wpool.tile([P, KC], f32, tag="loglam")
    nc.scalar.activation(loglam[:], lam[:], AF.Ln)

    # ---- combine and reduce ----
    per_event = wpool.tile([P, KC], f32, tag="per_event")
    nc.vector.tensor_tensor(out=per_event[:], in0=loglam[:], in1=negtot[:],
                            op=AL.add)
    colsum = wpool.tile([P, 1], f32, tag="colsum")
    nc.vector.tensor_reduce(out=colsum[:], in_=per_event[:], axis=AX.X,
                            op=AL.add)
    totp = ps.tile([1, 1], f32, tag="totp")
    nc.tensor.matmul(totp[:], ones_col[:], colsum[:], start=True, stop=True)
    final = spool.tile([1, 1], f32, tag="final")
    nc.vector.tensor_tensor(out=final[:], in0=totp[:], in1=negconst,
                            op=AL.add)
    nc.sync.dma_start(out=out_ap, in_=final[:])


_CACHE = {}


def _build(Tval: float):
    key = float(Tval)
    if key in _CACHE:
        return _CACHE[key]
    nc = bacc.Bacc("TRN2", target_bir_lowering=False, debug=False)
    ins = {}
    for name, (shape, dt) in INPUTS.items():
        ins[name] = nc.dram_tensor(name, list(shape), dt,
                                   kind="ExternalInput").ap()
    out_ap = nc.dram_tensor("out", [1, 1], f32, kind="ExternalOutput").ap()
    with tile.TileContext(nc) as tc:
        with ExitStack() as ctx:
            _body(ctx, tc, ins, out_ap, Tval)
    nc.compile()
    _CACHE[key] = (nc, ins, out_ap)
    return _CACHE[key]


def host_prep(mu_raw, log_alpha, log_beta, Tval):
    """O(D^2) parameter transforms in float64 -> float32."""
    mu = np.log1p(np.exp(np.float64(mu_raw))).astype(np.float32)
    al = np.log1p(np.exp(np.float64(log_alpha))).astype(np.float32)
    be = np.log1p(np.exp(np.float64(log_beta))).astype(np.float32)
    ab = (al * be).astype(np.float32)

    pack_bf = np.zeros((P, 748), dtype=ml_dtypes.bfloat16)
    pack_bf[:, 0:128] = np.triu(np.ones((P, P), dtype=np.float32))
    pack_bf[:, 128:228] = np.broadcast_to(ab.reshape(-1), (P, RM))
    pack_bf[:, 228:328] = np.broadcast_to(al.T.reshape(-1), (P, RM))
    pack_bf[:, 328:428] = np.broadcast_to(be.T.reshape(-1), (P, RM))
    pack_bf[:, 428:588] = np.tile(np.arange(D, dtype=np.float32), KC)[None, :]

    pf_const = np.zeros((P, 285), dtype=np.float32)
    pf_const[:, 48:148] = np.broadcast_to(be.reshape(-1), (P, RM))
    pf_const[:RM, 168:268] = np.eye(RM, dtype=np.float32)
    pf_const[0, 284] = np.float32(-Tval * mu.astype(np.float64).sum())

    tabs = np.zeros((D, 23), dtype=np.float32)
    tabs[:, 0:10] = be.T
    tabs[:, 10:20] = al.T
    tabs[:, 20] = mu - np.diag(ab)
    asum = al.sum(axis=0)
    asum_hi = asum.astype(ml_dtypes.bfloat16).astype(np.float32)
    tabs[:, 21] = asum_hi
    tabs[:, 22] = asum - asum_hi
    return pack_bf, pf_const, be, tabs


SELMASK = np.zeros((KC, KC, RM), dtype=ml_dtypes.bfloat16)
for _k in range(KC - 1):
    SELMASK[_k, _k + 1, :] = 1.0
SELMASK = SELMASK.reshape(KC, KC * RM)


def make_in_maps(time_points, event_types, mu_raw, log_alpha, log_beta, T):
    Tval = float(np.asarray(T))
    tp = np.asarray(time_points, dtype=np.float32)
    et = np.asarray(event_types).astype(np.float32)
    pack_bf, pf_const, be, tabs = host_prep(
        np.asarray(mu_raw), np.asarray(log_alpha), np.asarray(log_beta), Tval)
    in_maps = []
    for b in range(B):
        ts = tp[b, ::P]                       # [16] chunk reference times
        dtb = np.zeros(KC, dtype=np.float32)
        dtb[:-1] = ts[1:] - ts[:-1]
        pack_f32 = pf_const.copy()
        t2d = tp[b].reshape(KC, P).T
        pack_f32[:, 0:16] = t2d - ts[None, :]
        pack_f32[:, 16:32] = et[b].reshape(KC, P).T
        pack_f32[:, 32:48] = t2d - np.float32(Tval)
        pack_f32[:RM, 268:284] = be.reshape(-1)[:, None] * dtb[None, :]
        oht = np.zeros((D, N + 23), dtype=ml_dtypes.bfloat16)
        ohmat = (et[b][None, :] == np.arange(D, dtype=np.float32)[:, None])
        oht[:, 0:N] = ohmat
        oht[:, N:N + 23] = tabs
        pbb = pack_bf.copy()
        pbb[:, 588:748] = ohmat.T.astype(np.float32).reshape(
            KC, P, D).transpose(1, 0, 2).reshape(P, KC * D)
        in_maps.append({"pack_f32": pack_f32, "pack_bf": pbb, "oht": oht,
                        "selmask": SELMASK})
    return in_maps, Tval


def kernel(time_points, event_types, mu_raw, log_alpha, log_beta, T):
    in_maps, Tval = make_in_maps(time_points, event_types, mu_raw,
                                 log_alpha, log_beta, T)
    nc, _, _ = _build(Tval)
    res = run_bass_kernel_spmd(nc, in_maps, list(range(B))).results
    out = np.array([res[b]["out"][0, 0] for b in range(B)], dtype=np.float32)
    return out



# revision 2
# speedup vs baseline: 1.0650x; 1.0650x over previous
"""Trainium2 Bass kernel for the exp-kernel multivariate Hawkes process
log-likelihood (B=8, N=2048, D=10).

Strategy (v2)
-------------
Data-parallel over batch: core b computes batch row b fully on-chip and
returns a [1,16] row of per-chunk partial sums; the host adds the 16
values and the -T*sum(mu) constant.

Host ships per-event GATHERED tables (pure index lookups, O(N*D)):
  bcol[j,r]  = b[r, e_j]        brow_neg[j,m] = -b[e_j, m]
  acolT[j,d] = a[d, e_j]        abrow[j,m]    = (a*b)[e_j, m]
so the device only exponentiates [N,10] grids (not [N,100]):
  u[j,r]   = exp(bcol * trel_j)         (drives the pairwise kernel)
  vab[i,m] = exp(brow_neg * trel_i) * abrow
The [128, KC*100] pair grids are then single outer-product builds:
  W      = u  (x) onehot_m     (DVE)
  Mfull  = onehot_r (x) vab    (GPSIMD, off the critical path)
  lam_core = sum_rm Mfull * (triu @ W)      per chunk (prefix matmul)

Inter-chunk state S is kept in [10_r, (10_m, 16_k)] layout end-to-end:
chunk sums come from 16 tiny u^T@onehot matmuls written straight into
that layout, the affine recurrence S_{k+1}=d_k(S_k+w_k) is ONE
tensor_tensor_scan with a k=0 decay-reset column, and S is gathered
per event (S_k[e_i, m]) with 15 onehotT matmuls - no transpose, no
[KC,KC*RM] select-mask, no PSUM broadcast injection.

The Pg (prefix) PSUM tiles are copied to bf16 SBUF by the otherwise
idle Activation engine so the DVE tail products run in 2x mode.
A manually emitted InstLoadActFuncSet(id=6) loads the combined
exp+ln table once (the default pass would load two tables).

Host-side work is limited to O(D^2) parameter softplus and O(N*D)
gathers/reshapes of the index tensors (no transcendental math on N).
"""
import numpy as np
from contextlib import ExitStack

import ml_dtypes
import concourse.bass as bass
import concourse.mybir as mybir
import concourse.tile as tile
from concourse import bacc
from concourse.bass_utils import run_bass_kernel_spmd

f32 = mybir.dt.float32
bf16 = mybir.dt.bfloat16
AL = mybir.AluOpType
AF = mybir.ActivationFunctionType
AX = mybir.AxisListType

P = 128          # partitions == chunk size
KC = 16          # number of chunks
D = 10           # event types
RM = D * D       # (receiver, trigger) pairs
N = P * KC       # 2048 events per batch row
B = 8            # batch == cores
NG = 4           # chunk groups (4 chunks per PSUM bank)

# packed DRAM inputs: name -> (shape, dtype)
# hf: trel(16) tau2(16) bcol(160) brow_neg(160) musub_ev(16) asum_ev(16)
#     ones(1)
# hb: onehot(160) acolT(160) abrow(160) triu(128)
# oht: transposed onehot [D, N] for the per-event S gathers
# bdtbp: decay args [10, (m,k)] twice - true | k0-killed
INPUTS = {
    "hf": ((P, 385), f32),
    "hb": ((P, 608), bf16),
    "oht": ((D, N), bf16),
    "bdtbp": ((D, 320), f32),
}


def _body(ctx: ExitStack, tc, ins, out_ap):
    nc = tc.nc
    cpool = ctx.enter_context(tc.tile_pool(name="cpool", bufs=1))
    wpool = ctx.enter_context(tc.tile_pool(name="wpool", bufs=1))
    pp = ctx.enter_context(tc.tile_pool(name="pp", bufs=1, space="PSUM"))

    # ---- input DMAs on three parallel trigger queues ----
    hf = cpool.tile([P, 385], f32, tag="hf")
    nc.sync.dma_start(out=hf[:], in_=ins["hf"])
    hb = cpool.tile([P, 608], bf16, tag="hb")
    nc.scalar.dma_start(out=hb[:], in_=ins["hb"])
    oht = cpool.tile([D, N], bf16, tag="oht")
    nc.gpsimd.dma_start(out=oht[:], in_=ins["oht"])
    bdtbp = cpool.tile([D, 320], f32, tag="bdtbp")
    nc.gpsimd.dma_start(out=bdtbp[:], in_=ins["bdtbp"])

    # one combined exp+ln activation table load (id 6 =
    # natural_log_exp_and_others); the auto-pass then inserts none.
    nc.scalar.add_instruction(mybir.InstLoadActFuncSet(
        name=nc.get_next_instruction_name(), act_func_set_id=6,
        ins=[], outs=[]))

    trel = hf[:, 0:16]
    tau2 = hf[:, 16:32]
    bcol = hf[:, 32:192].rearrange("p (c r) -> p c r", c=KC)
    brow_neg = hf[:, 192:352].rearrange("p (c m) -> p c m", c=KC)
    musub_ev = hf[:, 352:368]
    asum_ev = hf[:, 368:384]
    ones_col = hf[:, 384:385]
    onehot = hb[:, 0:160].rearrange("p (c m) -> p c m", c=KC)
    acolT = hb[:, 160:320].rearrange("p (c d) -> p c d", c=KC)
    abrow = hb[:, 320:480].rearrange("p (c m) -> p c m", c=KC)
    triu = hb[:, 480:608]

    # ---- per-event 10-wide exp pipelines ----
    uarg = wpool.tile([P, KC, D], f32, tag="uarg")
    nc.vector.tensor_tensor(
        out=uarg[:], in0=bcol,
        in1=trel.unsqueeze(2).broadcast_to([P, KC, D]), op=AL.mult)
    u = wpool.tile([P, KC, D], bf16, tag="u")
    nc.scalar.activation(u[:], uarg[:], AF.Exp)

    varg = wpool.tile([P, KC, D], f32, tag="varg")
    nc.vector.tensor_tensor(
        out=varg[:], in0=brow_neg,
        in1=trel.unsqueeze(2).broadcast_to([P, KC, D]), op=AL.mult)
    expv = wpool.tile([P, KC, D], bf16, tag="expv")
    nc.scalar.activation(expv[:], varg[:], AF.Exp)
    vab = wpool.tile([P, KC, D], bf16, tag="vab")
    nc.vector.tensor_tensor(out=vab[:], in0=expv[:], in1=abrow, op=AL.mult)

    # ---- W grid: W[j,(r,m)] = u[j,r] * [e_j == m] ----
    W = wpool.tile([P, KC, D, D], bf16, tag="W")
    nc.vector.tensor_tensor(
        out=W[:],
        in0=u[:].unsqueeze(3).broadcast_to([P, KC, D, D]),
        in1=onehot.unsqueeze(2).broadcast_to([P, KC, D, D]), op=AL.mult)

    # ---- Mfull grid on GPSIMD: Mfull[i,(r,m)] = [e_i == r] * vab[i,m] ----
    Mfull = wpool.tile([P, KC, D, D], bf16, tag="Mfull")
    for g in range(NG):
        gs = slice(4 * g, 4 * (g + 1))
        nc.gpsimd.tensor_tensor(
            out=Mfull[:, gs],
            in0=onehot[:, gs].unsqueeze(3).broadcast_to([P, 4, D, D]),
            in1=vab[:, gs].unsqueeze(2).broadcast_to([P, 4, D, D]),
            op=AL.mult)

    # ---- chunk sums straight into scan layout: wsq[r, m, k] ----
    wsq = pp.tile([D, D, KC], f32, tag="wsq", name="wsq")
    for k in range(KC):
        nc.tensor.matmul(wsq[:, :, k], u[:, k, :], onehot[:, k, :],
                         start=True, stop=True)

    # ---- decays + affine scan over chunks ----
    # decays[:, 0:160] = exp(-b*dtb) true; [:, 160:320] same with k=0 -> 0
    decays = wpool.tile([D, 320], f32, tag="decays")
    nc.scalar.activation(decays[:], bdtbp[:], AF.Exp, scale=-1.0)
    dw = wpool.tile([D, D, KC], f32, tag="dw")
    nc.vector.tensor_tensor(
        out=dw[:], in0=decays[:, 0:160].rearrange("p (m k) -> p m k", m=D),
        in1=wsq[:], op=AL.mult)
    # S_{k+1} = d0op_k * S_k + d_k*w_k  (d0op kills state at k=0 per m)
    Sout = wpool.tile([D, D * KC], bf16, tag="Sout")
    nc.vector.tensor_tensor_scan(
        Sout[:], decays[:, 160:320],
        dw[:].rearrange("p m k -> p (m k)"), initial=0.0,
        op0=AL.mult, op1=AL.add)
    Soutv = Sout[:].rearrange("p (m k) -> p m k", m=D)

    # ---- in-chunk inclusive prefix (PE) ----
    Pg = [pp.tile([P, 4, D, D], f32, tag=f"Pg{g}", name=f"Pg{g}")
          for g in range(NG)]
    for g in range(NG):
        nc.tensor.matmul(Pg[g][:], triu,
                         W[:, 4 * g:4 * (g + 1)].rearrange(
                             "p c r m -> p (c r m)"),
                         start=True, stop=True)

    # ---- gather inter-chunk state per event: Sg[i,m] = S_k[e_i, m] ----
    Sgall = pp.tile([P, KC, D], f32, tag="Sgall", name="Sgall")
    nc.vector.memset(Sgall[:, 0:1, :], 0.0)
    for k in range(1, KC):
        nc.tensor.matmul(Sgall[:, k, :], oht[:, k * P:(k + 1) * P],
                         Soutv[:, :, k - 1], start=True, stop=True)

    # ---- tail: Act copies PSUM->SBUF bf16, DVE 2x products + reduces ----
    PgSB = wpool.tile([P, KC, D, D], bf16, tag="PgSB")
    G2 = wpool.tile([P, KC, D, D], bf16, tag="G2")
    lamr = wpool.tile([P, KC], f32, tag="lamr")
    for g in range(NG):
        gs = slice(4 * g, 4 * (g + 1))
        nc.scalar.copy(PgSB[:, gs], Pg[g][:])
        nc.vector.tensor_tensor(out=G2[:, gs], in0=PgSB[:, gs],
                                in1=Mfull[:, gs], op=AL.mult)
        nc.vector.tensor_reduce(
            out=lamr[:, gs],
            in_=G2[:, gs].rearrange("p c r m -> p c (r m)"),
            axis=AX.X, op=AL.add)

    # ---- negative (integral) part, mostly on GPSIMD ----
    argN = wpool.tile([P, KC, D], f32, tag="argN")
    nc.gpsimd.tensor_tensor(
        out=argN[:], in0=bcol,
        in1=tau2.unsqueeze(2).broadcast_to([P, KC, D]), op=AL.mult)
    expn = wpool.tile([P, KC, D], bf16, tag="expn")
    nc.scalar.activation(expn[:], argN[:], AF.Exp)
    nmul = wpool.tile([P, KC, D], bf16, tag="nmul")
    nc.gpsimd.tensor_tensor(out=nmul[:], in0=expn[:], in1=acolT, op=AL.mult)
    negred = wpool.tile([P, KC], f32, tag="negred")
    nc.vector.tensor_reduce(out=negred[:], in_=nmul[:], axis=AX.X, op=AL.add)

    # ---- S contribution + combine ----
    sg1 = wpool.tile([P, KC, D], f32, tag="sg1")
    nc.vector.tensor_tensor(out=sg1[:], in0=vab[:], in1=Sgall[:], op=AL.mult)
    sgr = wpool.tile([P, KC], f32, tag="sgr")
    nc.vector.tensor_reduce(out=sgr[:], in_=sg1[:], axis=AX.X, op=AL.add)

    lam1 = wpool.tile([P, KC], f32, tag="lam1")
    nc.vector.tensor_tensor(out=lam1[:], in0=lamr[:], in1=sgr[:], op=AL.add)
    lam = wpool.tile([P, KC], f32, tag="lam")
    nc.vector.tensor_tensor(out=lam[:], in0=lam1[:], in1=musub_ev,
                            op=AL.add)
    loglam = wpool.tile([P, KC], f32, tag="loglam")
    nc.scalar.activation(loglam[:], lam[:], AF.Ln)

    pt1 = wpool.tile([P, KC], f32, tag="pt1")
    nc.vector.tensor_tensor(out=pt1[:], in0=loglam[:], in1=negred[:],
                            op=AL.add)
    pev = wpool.tile([P, KC], f32, tag="pev")
    nc.vector.tensor_tensor(out=pev[:], in0=pt1[:], in1=asum_ev,
                            op=AL.subtract)

    # ---- partition reduce via PE, host adds the 16 values ----
    totp = pp.tile([1, KC], f32, tag="totp", name="totp")
    nc.tensor.matmul(totp[:], ones_col, pev[:], start=True, stop=True)
    outrow = wpool.tile([1, KC], f32, tag="outrow")
    nc.vector.tensor_copy(out=outrow[:], in_=totp[:])
    nc.sync.dma_start(out=out_ap, in_=outrow[:])


_CACHE = {}


def _build(Tval: float = 0.0):
    key = 0
    if key in _CACHE:
        return _CACHE[key]
    nc = bacc.Bacc("TRN2", target_bir_lowering=False, debug=False)
    ins = {}
    for name, (shape, dt) in INPUTS.items():
        ins[name] = nc.dram_tensor(name, list(shape), dt,
                                   kind="ExternalInput").ap()
    out_ap = nc.dram_tensor("out", [1, KC], f32, kind="ExternalOutput").ap()
    with tile.TileContext(nc) as tc:
        with ExitStack() as ctx:
            _body(ctx, tc, ins, out_ap)
    nc.compile()
    _CACHE[key] = (nc, ins, out_ap)
    return _CACHE[key]


def make_in_maps(time_points, event_types, mu_raw, log_alpha, log_beta, T):
    Tval = float(np.asarray(T))
    tp = np.asarray(time_points, dtype=np.float32)          # [B, N]
    et = np.asarray(event_types).astype(np.int64)           # [B, N]

    # O(D^2) parameter transforms in float64 -> float32
    mu = np.log1p(np.exp(np.float64(mu_raw))).astype(np.float32)
    al = np.log1p(np.exp(np.float64(log_alpha))).astype(np.float32)
    be = np.log1p(np.exp(np.float64(log_beta))).astype(np.float32)
    ab = (al * be).astype(np.float32)
    musub = mu - np.diag(ab)                                # [D]
    asum = al.sum(axis=0)                                   # [D]
    beT = np.ascontiguousarray(be.T)
    alT = np.ascontiguousarray(al.T)

    triu = np.triu(np.ones((P, P), dtype=np.float32))

    in_maps = []
    for b in range(B):
        e = et[b]                                           # [N]
        t = tp[b]
        ts = t[::P]                                         # [KC]
        dtb = np.zeros(KC, dtype=np.float32)
        dtb[:-1] = ts[1:] - ts[:-1]

        # [p, c] views (event j = c*128 + p)
        t2 = t.reshape(KC, P).T                             # [P, KC]
        e2 = e.reshape(KC, P).T                             # [P, KC]
        trel = t2 - ts[None, :]
        tau2 = t2 - np.float32(Tval)

        hf = np.zeros((P, 385), dtype=np.float32)
        hf[:, 0:16] = trel
        hf[:, 16:32] = tau2
        hf[:, 32:192] = beT[e2].reshape(P, KC * D)          # bcol
        hf[:, 192:352] = (-be)[e2].reshape(P, KC * D)       # brow_neg
        hf[:, 352:368] = musub[e2]
        hf[:, 368:384] = asum[e2]
        hf[:, 384] = 1.0

        hb = np.zeros((P, 608), dtype=ml_dtypes.bfloat16)
        oh = (e2[:, :, None] == np.arange(D)[None, None, :])
        hb[:, 0:160] = oh.reshape(P, KC * D)
        hb[:, 160:320] = alT[e2].reshape(P, KC * D)         # acolT
        hb[:, 320:480] = ab[e2].reshape(P, KC * D)          # abrow
        hb[:, 480:608] = triu

        oht = (e[None, :] == np.arange(D)[:, None]).astype(
            ml_dtypes.bfloat16)                             # [D, N]

        bdtb = be[:, :, None] * dtb[None, None, :]          # [D, D, KC]
        bdtbp = np.zeros((D, 320), dtype=np.float32)
        bdtbp[:, 0:160] = bdtb.reshape(D, D * KC)
        bk0 = bdtb.copy()
        bk0[:, :, 0] = 40.0                                 # exp(-40) ~ 0
        bdtbp[:, 160:320] = bk0.reshape(D, D * KC)

        in_maps.append({"hf": hf, "hb": hb, "oht": oht, "bdtbp": bdtbp})
    negconst = np.float32(-Tval * mu.astype(np.float64).sum())
    return in_maps, Tval, negconst


def kernel(time_points, event_types, mu_raw, log_alpha, log_beta, T):
    in_maps, Tval, negconst = make_in_maps(
        time_points, event_types, mu_raw, log_alpha, log_beta, T)
    nc, _, _ = _build(Tval)
    res = run_bass_kernel_spmd(nc, in_maps, list(range(B))).results
    out = np.array([res[b]["out"].sum() + negconst for b in range(B)],
                   dtype=np.float32)
    return out


# revision 4
# speedup vs baseline: 1.1154x; 1.0473x over previous
"""Trainium2 Bass kernel for the exp-kernel multivariate Hawkes process
log-likelihood (B=8, N=2048, D=10).

Strategy (v3)
-------------
Data-parallel over batch: core b computes batch row b fully on-chip and
returns a [1,16] row of per-chunk partial sums; the host adds the 16
values and the -T*sum(mu) constant.

Host ships per-event GATHERED tables (pure index lookups, O(N*D)):
  bcol[j,r]  = b[r, e_j]        brow_neg[j,m] = -b[e_j, m]
  acolT[j,d] = a[d, e_j]        abrow[j,m]    = (a*b)[e_j, m]
so the device only exponentiates [N,10] grids (not [N,100]):
  u[j,r]   = exp(bcol * trel_j)
  vab[i,m] = exp(brow_neg * trel_i) * abrow
One [128, KC*100] pair grid W = u (x) onehot_m feeds the in-chunk
inclusive-prefix matmuls (triu stationary).  The tail is FACTORED so no
second pair grid is needed:
  lam_core[i] = sum_r onehot_r[i,r] * (sum_m PgSB[i,(r,m)] * vab[i,m])
i.e. one 2x-mode product against the Act-engine bf16 copy of the PSUM
prefix, an X-reduce over m, a small r-mask, and an X-reduce over r.

Inter-chunk state S is kept in [10_r, (10_m, 16_k)] layout end-to-end:
chunk sums come from 16 tiny u^T@onehot matmuls written straight into
that layout (strided PSUM writes), the affine recurrence
S_{k+1}=d_k(S_k+w_k) is ONE tensor_tensor_scan (on GPSIMD) with a k=0
decay-reset column, and S is gathered per event (S_k[e_i,m]) with 15
onehotT matmuls.

Hot inputs (trel/bcol/onehot/triu) ride in the first two DMAs so the
exp pipeline starts as soon as the DMA window opens; everything else
follows on second triggers / the gpsimd queue.

Host-side work is limited to O(D^2) parameter softplus and O(N*D)
gathers/reshapes of the index tensors (no transcendental math on N).
"""
import numpy as np
from contextlib import ExitStack

import ml_dtypes
import concourse.bass as bass
import concourse.mybir as mybir
import concourse.tile as tile
from concourse import bacc
from concourse.bass_utils import run_bass_kernel_spmd

f32 = mybir.dt.float32
bf16 = mybir.dt.bfloat16
AL = mybir.AluOpType
AF = mybir.ActivationFunctionType
AX = mybir.AxisListType

P = 128          # partitions == chunk size
KC = 16          # number of chunks
D = 10           # event types
RM = D * D       # (receiver, trigger) pairs
N = P * KC       # 2048 events per batch row
B = 8            # batch == cores
NG = 4           # chunk groups (4 chunks per PSUM bank)

# packed DRAM inputs: name -> (shape, dtype)
INPUTS = {
    "hot_f32": ((P, 176), f32),    # trel(16) bcol(160)
    "hot_bf": ((P, 288), bf16),    # onehot(160) triu(128)
    "rest_f32": ((P, 209), f32),   # tau2(16) brow_neg(160) musub_ev(16)
                                   # asum_ev(16) ones(1)
    "rest_bf": ((P, 320), bf16),   # acolT(160) abrow(160)
    "oht": ((D, N), bf16),         # transposed onehot for the S gathers
    "bdtbp": ((D, 320), f32),      # decay args [10,(m,k)]: true | k0-killed
}


def _body(ctx: ExitStack, tc, ins, out_ap):
    nc = tc.nc
    cpool = ctx.enter_context(tc.tile_pool(name="cpool", bufs=1))
    wpool = ctx.enter_context(tc.tile_pool(name="wpool", bufs=1))
    pp = ctx.enter_context(tc.tile_pool(name="pp", bufs=1, space="PSUM"))

    # one combined exp+ln activation table load (id 6 =
    # natural_log_exp_and_others) emitted first on the Act queue
    nc.scalar.add_instruction(mybir.InstLoadActFuncSet(
        name=nc.get_next_instruction_name(), act_func_set_id=6,
        ins=[], outs=[]))

    # ---- input DMAs: hot tiles first on the sync/scalar queues ----
    hot_f32 = cpool.tile([P, 176], f32, tag="hot_f32")
    nc.sync.dma_start(out=hot_f32[:], in_=ins["hot_f32"])
    hot_bf = cpool.tile([P, 288], bf16, tag="hot_bf")
    nc.scalar.dma_start(out=hot_bf[:], in_=ins["hot_bf"])
    rest_f32 = cpool.tile([P, 209], f32, tag="rest_f32")
    nc.sync.dma_start(out=rest_f32[:], in_=ins["rest_f32"])
    rest_bf = cpool.tile([P, 320], bf16, tag="rest_bf")
    nc.scalar.dma_start(out=rest_bf[:], in_=ins["rest_bf"])
    oht = cpool.tile([D, N], bf16, tag="oht")
    nc.gpsimd.dma_start(out=oht[:], in_=ins["oht"])
    bdtbp = cpool.tile([D, 320], f32, tag="bdtbp")
    nc.gpsimd.dma_start(out=bdtbp[:], in_=ins["bdtbp"])

    trel = hot_f32[:, 0:16]
    bcol = hot_f32[:, 16:176].rearrange("p (c r) -> p c r", c=KC)
    onehot = hot_bf[:, 0:160].rearrange("p (c m) -> p c m", c=KC)
    triu = hot_bf[:, 160:288]
    tau2 = rest_f32[:, 0:16]
    brow_neg = rest_f32[:, 16:176].rearrange("p (c m) -> p c m", c=KC)
    musub_ev = rest_f32[:, 176:192]
    asum_ev = rest_f32[:, 192:208]
    ones_col = rest_f32[:, 208:209]
    acolT = rest_bf[:, 0:160].rearrange("p (c d) -> p c d", c=KC)
    abrow = rest_bf[:, 160:320].rearrange("p (c m) -> p c m", c=KC)

    # ---- critical chain: uarg -> u -> W -> prefix ----
    uarg = wpool.tile([P, KC, D], f32, tag="uarg")
    nc.vector.tensor_tensor(
        out=uarg[:], in0=bcol,
        in1=trel.unsqueeze(2).broadcast_to([P, KC, D]), op=AL.mult)
    u = wpool.tile([P, KC, D], bf16, tag="u")
    nc.scalar.activation(u[:], uarg[:], AF.Exp)
    W = wpool.tile([P, KC, D, D], bf16, tag="W")
    nc.vector.tensor_tensor(
        out=W[:],
        in0=u[:].unsqueeze(3).broadcast_to([P, KC, D, D]),
        in1=onehot.unsqueeze(2).broadcast_to([P, KC, D, D]), op=AL.mult)

    # chunk sums straight into scan layout: wsq[r, m, k]
    wsq = pp.tile([D, D, KC], f32, tag="wsq", name="wsq")
    for k in range(KC):
        nc.tensor.matmul(wsq[:, :, k], u[:, k, :], onehot[:, k, :],
                         start=True, stop=True)

    # ---- secondary exp pipelines ----
    varg = wpool.tile([P, KC, D], f32, tag="varg")
    nc.vector.tensor_tensor(
        out=varg[:], in0=brow_neg,
        in1=trel.unsqueeze(2).broadcast_to([P, KC, D]), op=AL.mult)
    expv = wpool.tile([P, KC, D], bf16, tag="expv")
    nc.scalar.activation(expv[:], varg[:], AF.Exp)
    argN = wpool.tile([P, KC, D], f32, tag="argN")
    nc.vector.tensor_tensor(
        out=argN[:], in0=bcol,
        in1=tau2.unsqueeze(2).broadcast_to([P, KC, D]), op=AL.mult)
    expn = wpool.tile([P, KC, D], bf16, tag="expn")
    nc.scalar.activation(expn[:], argN[:], AF.Exp)
    vab = wpool.tile([P, KC, D], bf16, tag="vab")
    nc.vector.tensor_tensor(out=vab[:], in0=expv[:], in1=abrow, op=AL.mult)
    nmul = wpool.tile([P, KC, D], bf16, tag="nmul")
    nc.vector.tensor_tensor(out=nmul[:], in0=expn[:], in1=acolT, op=AL.mult)
    negred = wpool.tile([P, KC], f32, tag="negred")
    nc.vector.tensor_reduce(out=negred[:], in_=nmul[:], axis=AX.X, op=AL.add)

    # ---- decays + affine scan over chunks (scan on GPSIMD) ----
    decays = wpool.tile([D, 320], f32, tag="decays")
    nc.scalar.activation(decays[:], bdtbp[:], AF.Exp, scale=-1.0)
    dw = wpool.tile([D, D, KC], f32, tag="dw")
    nc.vector.tensor_tensor(
        out=dw[:], in0=decays[:, 0:160].rearrange("p (m k) -> p m k", m=D),
        in1=wsq[:], op=AL.mult)
    # S_{k+1} = d0op_k * S_k + d_k*w_k  (d0op kills state at k=0 per m)
    Sout = wpool.tile([D, D * KC], bf16, tag="Sout")
    nc.vector.tensor_tensor_scan(
        Sout[:], decays[:, 160:320],
        dw[:].rearrange("p m k -> p (m k)"), initial=0.0,
        op0=AL.mult, op1=AL.add)
    Soutv = Sout[:].rearrange("p (m k) -> p m k", m=D)

    # ---- in-chunk inclusive prefix (PE) ----
    Pg = [pp.tile([P, 4, D, D], f32, tag=f"Pg{g}", name=f"Pg{g}")
          for g in range(NG)]
    for g in range(NG):
        nc.tensor.matmul(Pg[g][:], triu,
                         W[:, 4 * g:4 * (g + 1)].rearrange(
                             "p c r m -> p (c r m)"),
                         start=True, stop=True)

    # ---- gather inter-chunk state per event: Sg[i,m] = S_k[e_i, m] ----
    Sgall = pp.tile([P, KC, D], f32, tag="Sgall", name="Sgall")
    nc.vector.memset(Sgall[:, 0:1, :], 0.0)
    for k in range(1, KC):
        nc.tensor.matmul(Sgall[:, k, :], oht[:, k * P:(k + 1) * P],
                         Soutv[:, :, k - 1], start=True, stop=True)

    # ---- tail: Act copies PSUM->SBUF bf16, factored product/reduce ----
    PgSB = wpool.tile([P, KC, D, D], bf16, tag="PgSB")
    for g in range(NG):
        nc.scalar.copy(PgSB[:, 4 * g:4 * (g + 1)], Pg[g][:])
    t1 = wpool.tile([P, KC, D, D], bf16, tag="t1")
    Q = wpool.tile([P, KC, D], f32, tag="Q")
    for h in range(2):
        hs = slice(8 * h, 8 * (h + 1))
        nc.vector.tensor_tensor(
            out=t1[:, hs], in0=PgSB[:, hs],
            in1=vab[:, hs].unsqueeze(2).broadcast_to([P, 8, D, D]),
            op=AL.mult)
        nc.vector.tensor_reduce(out=Q[:, hs], in_=t1[:, hs],
                                axis=AX.X, op=AL.add)
    t2 = wpool.tile([P, KC, D], f32, tag="t2")
    nc.vector.tensor_tensor(out=t2[:], in0=Q[:], in1=onehot, op=AL.mult)
    lamr = wpool.tile([P, KC], f32, tag="lamr")
    nc.vector.tensor_reduce(out=lamr[:], in_=t2[:], axis=AX.X, op=AL.add)

    # ---- S contribution + combine ----
    sg1 = wpool.tile([P, KC, D], f32, tag="sg1")
    nc.vector.tensor_tensor(out=sg1[:], in0=vab[:], in1=Sgall[:], op=AL.mult)
    sgr = wpool.tile([P, KC], f32, tag="sgr")
    nc.vector.tensor_reduce(out=sgr[:], in_=sg1[:], axis=AX.X, op=AL.add)

    lam1 = wpool.tile([P, KC], f32, tag="lam1")
    nc.vector.tensor_tensor(out=lam1[:], in0=lamr[:], in1=sgr[:], op=AL.add)
    lam = wpool.tile([P, KC], f32, tag="lam")
    nc.vector.tensor_tensor(out=lam[:], in0=lam1[:], in1=musub_ev,
                            op=AL.add)
    loglam = wpool.tile([P, KC], f32, tag="loglam")
    nc.scalar.activation(loglam[:], lam[:], AF.Ln)

    pt1 = wpool.tile([P, KC], f32, tag="pt1")
    nc.vector.tensor_tensor(out=pt1[:], in0=loglam[:], in1=negred[:],
                            op=AL.add)
    pev = wpool.tile([P, KC], f32, tag="pev")
    nc.vector.tensor_tensor(out=pev[:], in0=pt1[:], in1=asum_ev,
                            op=AL.subtract)

    # ---- partition reduce via PE, host adds the 16 values ----
    totp = pp.tile([1, KC], f32, tag="totp", name="totp")
    nc.tensor.matmul(totp[:], ones_col, pev[:], start=True, stop=True)
    outrow = wpool.tile([1, KC], f32, tag="outrow")
    nc.vector.tensor_copy(out=outrow[:], in_=totp[:])
    nc.sync.dma_start(out=out_ap, in_=outrow[:])


_CACHE = {}


def _build(Tval: float = 0.0):
    key = 0
    if key in _CACHE:
        return _CACHE[key]
    nc = bacc.Bacc("TRN2", target_bir_lowering=False, debug=False)
    ins = {}
    for name, (shape, dt) in INPUTS.items():
        ins[name] = nc.dram_tensor(name, list(shape), dt,
                                   kind="ExternalInput").ap()
    out_ap = nc.dram_tensor("out", [1, KC], f32, kind="ExternalOutput").ap()
    with tile.TileContext(nc) as tc:
        with ExitStack() as ctx:
            _body(ctx, tc, ins, out_ap)
    nc.compile()
    _CACHE[key] = (nc, ins, out_ap)
    return _CACHE[key]


def make_in_maps(time_points, event_types, mu_raw, log_alpha, log_beta, T):
    Tval = float(np.asarray(T))
    tp = np.asarray(time_points, dtype=np.float32)          # [B, N]
    et = np.asarray(event_types).astype(np.int64)           # [B, N]

    # O(D^2) parameter transforms in float64 -> float32
    mu = np.log1p(np.exp(np.float64(mu_raw))).astype(np.float32)
    al = np.log1p(np.exp(np.float64(log_alpha))).astype(np.float32)
    be = np.log1p(np.exp(np.float64(log_beta))).astype(np.float32)
    ab = (al * be).astype(np.float32)
    musub = mu - np.diag(ab)                                # [D]
    asum = al.sum(axis=0)                                   # [D]
    beT = np.ascontiguousarray(be.T)
    alT = np.ascontiguousarray(al.T)

    triu = np.triu(np.ones((P, P), dtype=np.float32))

    in_maps = []
    for b in range(B):
        e = et[b]                                           # [N]
        t = tp[b]
        ts = t[::P]                                         # [KC]
        dtb = np.zeros(KC, dtype=np.float32)
        dtb[:-1] = ts[1:] - ts[:-1]

        # [p, c] views (event j = c*128 + p)
        t2 = t.reshape(KC, P).T                             # [P, KC]
        e2 = e.reshape(KC, P).T                             # [P, KC]

        hot_f32 = np.zeros((P, 176), dtype=np.float32)
        hot_f32[:, 0:16] = t2 - ts[None, :]                 # trel
        hot_f32[:, 16:176] = beT[e2].reshape(P, KC * D)     # bcol

        oh = (e2[:, :, None] == np.arange(D)[None, None, :])
        hot_bf = np.zeros((P, 288), dtype=ml_dtypes.bfloat16)
        hot_bf[:, 0:160] = oh.reshape(P, KC * D)
        hot_bf[:, 160:288] = triu

        rest_f32 = np.zeros((P, 209), dtype=np.float32)
        rest_f32[:, 0:16] = t2 - np.float32(Tval)           # tau2
        rest_f32[:, 16:176] = (-be)[e2].reshape(P, KC * D)  # brow_neg
        rest_f32[:, 176:192] = musub[e2]
        rest_f32[:, 192:208] = asum[e2]
        rest_f32[:, 208] = 1.0

        rest_bf = np.zeros((P, 320), dtype=ml_dtypes.bfloat16)
        rest_bf[:, 0:160] = alT[e2].reshape(P, KC * D)      # acolT
        rest_bf[:, 160:320] = ab[e2].reshape(P, KC * D)     # abrow

        oht = (e[None, :] == np.arange(D)[:, None]).astype(
            ml_dtypes.bfloat16)                             # [D, N]

        bdtb = be[:, :, None] * dtb[None, None, :]          # [D, D, KC]
        bdtbp = np.zeros((D, 320), dtype=np.float32)
        bdtbp[:, 0:160] = bdtb.reshape(D, D * KC)
        bk0 = bdtb.copy()
        bk0[:, :, 0] = 40.0                                 # exp(-40) ~ 0
        bdtbp[:, 160:320] = bk0.reshape(D, D * KC)

        in_maps.append({"hot_f32": hot_f32, "hot_bf": hot_bf,
                        "rest_f32": rest_f32, "rest_bf": rest_bf,
                        "oht": oht, "bdtbp": bdtbp})
    negconst = np.float32(-Tval * mu.astype(np.float64).sum())
    return in_maps, Tval, negconst


def kernel(time_points, event_types, mu_raw, log_alpha, log_beta, T):
    in_maps, Tval, negconst = make_in_maps(
        time_points, event_types, mu_raw, log_alpha, log_beta, T)
    nc, _, _ = _build(Tval)
    res = run_bass_kernel_spmd(nc, in_maps, list(range(B))).results
    out = np.array([res[b]["out"].sum() + negconst for b in range(B)],
                   dtype=np.float32)
    return out


# revision 5
# speedup vs baseline: 1.1510x; 1.0320x over previous
"""Trainium2 Bass kernel for the exp-kernel multivariate Hawkes process
log-likelihood (B=8, N=2048, D=10).

Strategy (v4)
-------------
Data-parallel over batch: core b computes batch row b fully on-chip and
returns pev[128,16] per-event partials; the host reduces them and adds
the -T*sum(mu) constant (unshard step).

Host ships per-event GATHERED tables (pure index lookups, O(N*D)):
  bcol[j,r]  = b[r, e_j]        brow_neg[j,m] = -b[e_j, m]
  acolT[j,d] = a[d, e_j]        abrow[j,m]    = (a*b)[e_j, m]
so the device only exponentiates [N,10] grids:
  [u|expn]  = exp([bcol*trel | bcol*tau2])   (one fused DVE op + one Act op)
  vab[i,m]  = exp(brow_neg * trel_i) * abrow
One [128, KC*100] pair grid W = u (x) onehot_m (built in two halves to
overlap the prefix matmuls) feeds the in-chunk inclusive-prefix
(triu stationary).  The tail is FACTORED - no second pair grid:
  lam_core[i] = sum_r onehot_r[i,r] * (sum_m PgSB[i,(r,m)] * vab[i,m])
via per-group 2x products against the Act-engine bf16 copies of the
PSUM prefix + X-reduces, pipelined group-by-group against the copies.
The inter-chunk S contribution and the r-mask contraction share one
concatenated [P,KC,2,D] pass.

Inter-chunk state S lives in [10_r, (10_m, 16_k)] layout end-to-end:
chunk sums from 16 tiny u^T@onehot matmuls (strided PSUM writes), the
affine recurrence S_{k+1}=d_k(S_k+w_k) is ONE tensor_tensor_scan with
a k=0 decay-reset column, and S is gathered per event with 15 onehotT
matmuls.  A manually emitted InstLoadActFuncSet(id=6) loads the
combined exp+ln table once.

Host-side work is limited to O(D^2) parameter softplus and O(N*D)
gathers/reshapes/sums of index tensors (no transcendental math on N).
"""
import numpy as np
from contextlib import ExitStack

import ml_dtypes
import concourse.bass as bass
import concourse.mybir as mybir
import concourse.tile as tile
from concourse import bacc
from concourse.bass_utils import run_bass_kernel_spmd

f32 = mybir.dt.float32
bf16 = mybir.dt.bfloat16
AL = mybir.AluOpType
AF = mybir.ActivationFunctionType
AX = mybir.AxisListType

P = 128          # partitions == chunk size
KC = 16          # number of chunks
D = 10           # event types
RM = D * D       # (receiver, trigger) pairs
N = P * KC       # 2048 events per batch row
B = 8            # batch == cores
NG = 4           # chunk groups (4 chunks per PSUM bank)

# packed DRAM inputs: name -> (shape, dtype)
INPUTS = {
    "hot_f32": ((P, 192), f32),    # trel(16) tau2(16) bcol(160)
    "hot_bf": ((P, 288), bf16),    # onehot(160) triu(128)
    "rest_f32": ((P, 208), f32),   # brow_neg(160) musub_ev(16) asum_ev(16)
                                   # pad(16)
    "rest_bf": ((P, 640), bf16),   # acolT(160) abrow(160) ohone(320)
    "oht": ((D, N), bf16),         # transposed onehot for the S gathers
    "bdtbp": ((D, 320), f32),      # decay args [10,(m,k)]: true | k0-killed
}


def _body(ctx: ExitStack, tc, ins, out_ap):
    nc = tc.nc
    cpool = ctx.enter_context(tc.tile_pool(name="cpool", bufs=1))
    wpool = ctx.enter_context(tc.tile_pool(name="wpool", bufs=1))
    pp = ctx.enter_context(tc.tile_pool(name="pp", bufs=1, space="PSUM"))

    # one combined exp+ln activation table load (id 6 =
    # natural_log_exp_and_others) emitted first on the Act queue
    nc.scalar.add_instruction(mybir.InstLoadActFuncSet(
        name=nc.get_next_instruction_name(), act_func_set_id=6,
        ins=[], outs=[]))

    # ---- input DMAs: hot tiles first on the sync/scalar queues ----
    hot_f32 = cpool.tile([P, 192], f32, tag="hot_f32")
    nc.sync.dma_start(out=hot_f32[:], in_=ins["hot_f32"])
    hot_bf = cpool.tile([P, 288], bf16, tag="hot_bf")
    nc.scalar.dma_start(out=hot_bf[:], in_=ins["hot_bf"])
    rest_f32 = cpool.tile([P, 208], f32, tag="rest_f32")
    nc.sync.dma_start(out=rest_f32[:], in_=ins["rest_f32"])
    rest_bf = cpool.tile([P, 640], bf16, tag="rest_bf")
    nc.scalar.dma_start(out=rest_bf[:], in_=ins["rest_bf"])
    oht = cpool.tile([D, N], bf16, tag="oht")
    nc.gpsimd.dma_start(out=oht[:], in_=ins["oht"])
    bdtbp = cpool.tile([D, 320], f32, tag="bdtbp")
    nc.gpsimd.dma_start(out=bdtbp[:], in_=ins["bdtbp"])

    trel_tau = hot_f32[:, 0:32]
    bcol = hot_f32[:, 32:192].rearrange("p (c r) -> p c r", c=KC)
    onehot = hot_bf[:, 0:160].rearrange("p (c m) -> p c m", c=KC)
    triu = hot_bf[:, 160:288]
    trel = trel_tau[:, 0:16]
    brow_neg = rest_f32[:, 0:160].rearrange("p (c m) -> p c m", c=KC)
    musub_ev = rest_f32[:, 160:176]
    asum_ev = rest_f32[:, 176:192]
    acolT = rest_bf[:, 0:160].rearrange("p (c d) -> p c d", c=KC)
    abrow = rest_bf[:, 160:320].rearrange("p (c m) -> p c m", c=KC)
    ohone = rest_bf[:, 320:640].rearrange("p (c s m) -> p c s m", c=KC, s=2)

    # ---- fused exp args: au = [bcol*trel | bcol*tau2] ----
    au = wpool.tile([P, 2, KC, D], f32, tag="au")
    nc.vector.tensor_tensor(
        out=au[:],
        in0=bcol.unsqueeze(1).broadcast_to([P, 2, KC, D]),
        in1=trel_tau.rearrange("p (s c) -> p s c", s=2).unsqueeze(3)
            .broadcast_to([P, 2, KC, D]),
        op=AL.mult)
    eu = wpool.tile([P, 2, KC, D], bf16, tag="eu")
    nc.scalar.activation(eu[:], au[:], AF.Exp)
    u = eu[:, 0]
    expn = eu[:, 1]

    # ---- W grid in two halves: W[j,(r,m)] = u[j,r] * [e_j == m] ----
    W = wpool.tile([P, KC, D, D], bf16, tag="W")
    for h in range(2):
        hs = slice(8 * h, 8 * (h + 1))
        nc.vector.tensor_tensor(
            out=W[:, hs],
            in0=u[:, hs].unsqueeze(3).broadcast_to([P, 8, D, D]),
            in1=onehot[:, hs].unsqueeze(2).broadcast_to([P, 8, D, D]),
            op=AL.mult)

    # chunk sums straight into scan layout: wsq[r, m, k]
    wsq = pp.tile([D, D, KC], f32, tag="wsq", name="wsq")
    for k in range(KC):
        nc.tensor.matmul(wsq[:, :, k], u[:, k, :], onehot[:, k, :],
                         start=True, stop=True)

    # ---- secondary exp pipeline ----
    varg = wpool.tile([P, KC, D], f32, tag="varg")
    nc.vector.tensor_tensor(
        out=varg[:], in0=brow_neg,
        in1=trel.unsqueeze(2).broadcast_to([P, KC, D]), op=AL.mult)
    expv = wpool.tile([P, KC, D], bf16, tag="expv")
    nc.scalar.activation(expv[:], varg[:], AF.Exp)
    vab = wpool.tile([P, KC, D], bf16, tag="vab")
    nc.vector.tensor_tensor(out=vab[:], in0=expv[:], in1=abrow, op=AL.mult)

    # ---- decays + affine scan over chunks ----
    decays = wpool.tile([D, 320], f32, tag="decays")
    nc.scalar.activation(decays[:], bdtbp[:], AF.Exp, scale=-1.0)
    dw = wpool.tile([D, D, KC], f32, tag="dw")
    nc.vector.tensor_tensor(
        out=dw[:], in0=decays[:, 0:160].rearrange("p (m k) -> p m k", m=D),
        in1=wsq[:], op=AL.mult)
    # S_{k+1} = d0op_k * S_k + d_k*w_k  (d0op kills state at k=0 per m)
    Sout = wpool.tile([D, D * KC], bf16, tag="Sout")
    nc.vector.tensor_tensor_scan(
        Sout[:], decays[:, 160:320],
        dw[:].rearrange("p m k -> p (m k)"), initial=0.0,
        op0=AL.mult, op1=AL.add)
    Soutv = Sout[:].rearrange("p (m k) -> p m k", m=D)

    # ---- negative (integral) part ----
    nmul = wpool.tile([P, KC, D], bf16, tag="nmul")
    nc.vector.tensor_tensor(out=nmul[:], in0=expn, in1=acolT, op=AL.mult)
    negred = wpool.tile([P, KC], f32, tag="negred")
    nc.vector.tensor_reduce(out=negred[:], in_=nmul[:], axis=AX.X, op=AL.add)
    negsub = wpool.tile([P, KC], f32, tag="negsub")
    nc.vector.tensor_tensor(out=negsub[:], in0=negred[:], in1=asum_ev,
                            op=AL.subtract)

    # ---- in-chunk inclusive prefix (PE) ----
    Pg = [pp.tile([P, 4, D, D], f32, tag=f"Pg{g}", name=f"Pg{g}")
          for g in range(NG)]
    for g in range(NG):
        nc.tensor.matmul(Pg[g][:], triu,
                         W[:, 4 * g:4 * (g + 1)].rearrange(
                             "p c r m -> p (c r m)"),
                         start=True, stop=True)

    # ---- gather inter-chunk state per event: Sg[i,m] = S_k[e_i, m] ----
    Sgall = pp.tile([P, KC, D], f32, tag="Sgall", name="Sgall")
    nc.vector.memset(Sgall[:, 0:1, :], 0.0)
    for k in range(1, KC):
        nc.tensor.matmul(Sgall[:, k, :], oht[:, k * P:(k + 1) * P],
                         Soutv[:, :, k - 1], start=True, stop=True)

    # ---- tail: per-group Act copy -> 2x product -> X-reduce over m ----
    PgSB = wpool.tile([P, KC, D, D], bf16, tag="PgSB")
    t1 = wpool.tile([P, KC, D, D], bf16, tag="t1")
    QS = wpool.tile([P, KC, 2, D], f32, tag="QS")
    for g in range(NG):
        gs = slice(4 * g, 4 * (g + 1))
        nc.scalar.copy(PgSB[:, gs], Pg[g][:])
        nc.vector.tensor_tensor(
            out=t1[:, gs], in0=PgSB[:, gs],
            in1=vab[:, gs].unsqueeze(2).broadcast_to([P, 4, D, D]),
            op=AL.mult)
        nc.vector.tensor_reduce(out=QS[:, gs, 0, :], in_=t1[:, gs],
                                axis=AX.X, op=AL.add)
    # S contribution into the second lane of QS
    nc.vector.tensor_tensor(out=QS[:, :, 1, :], in0=vab[:], in1=Sgall[:],
                            op=AL.mult)
    # mask r-lane by onehot_r, S-lane by ones, contract both at once
    t2 = wpool.tile([P, KC, 2, D], f32, tag="t2")
    nc.vector.tensor_tensor(out=t2[:], in0=QS[:], in1=ohone, op=AL.mult)
    lamr = wpool.tile([P, KC], f32, tag="lamr")
    nc.vector.tensor_reduce(
        out=lamr[:], in_=t2[:].rearrange("p c s m -> p c (s m)"),
        axis=AX.X, op=AL.add)

    lam = wpool.tile([P, KC], f32, tag="lam")
    nc.vector.tensor_tensor(out=lam[:], in0=lamr[:], in1=musub_ev,
                            op=AL.add)
    loglam = wpool.tile([P, KC], f32, tag="loglam")
    nc.scalar.activation(loglam[:], lam[:], AF.Ln)
    pev = wpool.tile([P, KC], f32, tag="pev")
    nc.vector.tensor_tensor(out=pev[:], in0=loglam[:], in1=negsub[:],
                            op=AL.add)
    nc.sync.dma_start(out=out_ap, in_=pev[:])


_CACHE = {}


def _build(Tval: float = 0.0):
    key = 0
    if key in _CACHE:
        return _CACHE[key]
    nc = bacc.Bacc("TRN2", target_bir_lowering=False, debug=False)
    ins = {}
    for name, (shape, dt) in INPUTS.items():
        ins[name] = nc.dram_tensor(name, list(shape), dt,
                                   kind="ExternalInput").ap()
    out_ap = nc.dram_tensor("out", [P, KC], f32, kind="ExternalOutput").ap()
    with tile.TileContext(nc) as tc:
        with ExitStack() as ctx:
            _body(ctx, tc, ins, out_ap)
    nc.compile()
    _CACHE[key] = (nc, ins, out_ap)
    return _CACHE[key]


def make_in_maps(time_points, event_types, mu_raw, log_alpha, log_beta, T):
    Tval = float(np.asarray(T))
    tp = np.asarray(time_points, dtype=np.float32)          # [B, N]
    et = np.asarray(event_types).astype(np.int64)           # [B, N]

    # O(D^2) parameter transforms in float64 -> float32
    mu = np.log1p(np.exp(np.float64(mu_raw))).astype(np.float32)
    al = np.log1p(np.exp(np.float64(log_alpha))).astype(np.float32)
    be = np.log1p(np.exp(np.float64(log_beta))).astype(np.float32)
    ab = (al * be).astype(np.float32)
    musub = mu - np.diag(ab)                                # [D]
    asum = al.sum(axis=0)                                   # [D]
    beT = np.ascontiguousarray(be.T)
    alT = np.ascontiguousarray(al.T)

    triu = np.triu(np.ones((P, P), dtype=np.float32))

    in_maps = []
    for b in range(B):
        e = et[b]                                           # [N]
        t = tp[b]
        ts = t[::P]                                         # [KC]
        dtb = np.zeros(KC, dtype=np.float32)
        dtb[:-1] = ts[1:] - ts[:-1]

        # [p, c] views (event j = c*128 + p)
        t2 = t.reshape(KC, P).T                             # [P, KC]
        e2 = e.reshape(KC, P).T                             # [P, KC]

        hot_f32 = np.zeros((P, 192), dtype=np.float32)
        hot_f32[:, 0:16] = t2 - ts[None, :]                 # trel
        hot_f32[:, 16:32] = t2 - np.float32(Tval)           # tau2
        hot_f32[:, 32:192] = beT[e2].reshape(P, KC * D)     # bcol

        oh = (e2[:, :, None] == np.arange(D)[None, None, :])
        hot_bf = np.zeros((P, 288), dtype=ml_dtypes.bfloat16)
        hot_bf[:, 0:160] = oh.reshape(P, KC * D)
        hot_bf[:, 160:288] = triu

        rest_f32 = np.zeros((P, 208), dtype=np.float32)
        rest_f32[:, 0:160] = (-be)[e2].reshape(P, KC * D)   # brow_neg
        rest_f32[:, 160:176] = musub[e2]
        rest_f32[:, 176:192] = asum[e2]

        rest_bf = np.zeros((P, 640), dtype=ml_dtypes.bfloat16)
        rest_bf[:, 0:160] = alT[e2].reshape(P, KC * D)      # acolT
        rest_bf[:, 160:320] = ab[e2].reshape(P, KC * D)     # abrow
        ohone = np.zeros((P, KC, 2, D), dtype=np.float32)
        ohone[:, :, 0, :] = oh
        ohone[:, :, 1, :] = 1.0
        rest_bf[:, 320:640] = ohone.reshape(P, 320)

        oht = (e[None, :] == np.arange(D)[:, None]).astype(
            ml_dtypes.bfloat16)                             # [D, N]

        bdtb = be[:, :, None] * dtb[None, None, :]          # [D, D, KC]
        bdtbp = np.zeros((D, 320), dtype=np.float32)
        bdtbp[:, 0:160] = bdtb.reshape(D, D * KC)
        bk0 = bdtb.copy()
        bk0[:, :, 0] = 40.0                                 # exp(-40) ~ 0
        bdtbp[:, 160:320] = bk0.reshape(D, D * KC)

        in_maps.append({"hot_f32": hot_f32, "hot_bf": hot_bf,
                        "rest_f32": rest_f32, "rest_bf": rest_bf,
                        "oht": oht, "bdtbp": bdtbp})
    negconst = np.float32(-Tval * mu.astype(np.float64).sum())
    return in_maps, Tval, negconst


def kernel(time_points, event_types, mu_raw, log_alpha, log_beta, T):
    in_maps, Tval, negconst = make_in_maps(
        time_points, event_types, mu_raw, log_alpha, log_beta, T)
    nc, _, _ = _build(Tval)
    res = run_bass_kernel_spmd(nc, in_maps, list(range(B))).results
    out = np.array([res[b]["out"].sum() + negconst for b in range(B)],
                   dtype=np.float32)
    return out
